# revision 74
# baseline (speedup 1.0000x reference)
"""Trainium2 Bass kernel for nn_AttentionBlock (GroupNorm + single-head attention + residual).

Reference computation (b=4, c=256, h=w=64, n=h*w=4096):
    xn = GroupNorm(x, groups=8) * gamma + beta          # [b,c,n]
    q/k/v = w{q,k,v} @ xn + b{q,k,v}                    # 1x1 conv = channel matmul
    S = (q^T k) / sqrt(c);  P = softmax(S, axis=-1)     # [b,n,n]
    out = wp @ (v @ P^T) + bp + x

Sharding: pure data parallel, no collectives. Core p = 2*b + h handles batch b
and query half h (2048 queries), computing GroupNorm stats + keys/values for
its batch redundantly with its pair core. Each core returns y = out[b][:, half].

Math restructure:
  - GN fold: xn = A*x + B per channel (A = rstd*gamma, B = beta - mean*A).
  - S = xn_q^T M2 xn_k with M2 = wq^T wk. Key-side additive constants drop
    out of the softmax exactly (uniform per-query shift); bq == 0.
    KS = (M2^T . A) @ x, so S = KS^T xn_q -- no Q needed.
  - The four big attention matmuls (S, PV, softmax-denominator, projection)
    run in fp8e4m3 with MatmulPerfMode.DoubleRow: 2 fp8 rows/cycle and both
    128-deep contraction halves in one instruction -> 4x the fp32r rate.
    exp(s/16 - 3) keeps P inside fp8 range (max |s|/16 ~ 7.6; e^4.6=101 <
    240); the -3 shift cancels in the softmax. fp8 quantization noise lands
    on the attention branch only (~2.6% of the output after the residual):
    measured end-to-end rel err ~5e-3 vs the 2e-2 gate.
  - Softmax denominator on the PE: a DoubleRow matmul with an all-(1/32)
    fp8 stationary both reduces over keys and broadcasts den/32 across all
    128 partitions; a full-width [128,512] DVE reciprocal then gives
    32/den, so the normalize multiply feeds the fp8 projection at a good
    scale (attn*32 ~ N(0,0.83^2)) and a single scalar_tensor_tensor fuses
    the /32, deferred bias (pre-added into the residual) and residual add.
  - v bias deferral: cbv = wv@B + bv is pushed through the projection into
    cbp = wp@cbv + bp, which is pre-added into the residual copy.
"""

import numpy as np

P = 128
C = 256
HW = 4096
NQ = 2048
G = 8
EPS = 1e-5
NCORES = 8
QB = 512           # query block
NMB = HW // P      # 32 key chunks of 128
NPAIR = NMB // 2   # 16 key-chunk pairs per query block
NQB = NQ // QB     # 4 query blocks
SHIFT = -3.0       # exp bias: cancels in softmax, keeps P in fp8e4m3 range

_cache = {}


def _pack_consts(gamma, beta, bv, bp):
    """One packed [128, 24] tile: gamma/beta/bv/bp (chunked by 128) and the
    group-indicator matrix (value 1/32, block-diagonal over 32-channel groups)."""
    cst = np.zeros((P, 24), np.float32)
    for i, v in enumerate((gamma, beta, bv, bp)):
        cst[:, 2 * i:2 * i + 2] = np.asarray(v, np.float32).reshape(2, P).T
    for cc in range(2):
        for j in range(4):
            cst[32 * j:32 * (j + 1), 8 + cc * G + 4 * cc + j] = 1.0 / 32.0
    return cst


def _ones8():
    import ml_dtypes
    return np.full((P, 2, P), 1.0 / 32.0, ml_dtypes.float8_e4m3)


def _build():
    import concourse.bass as bass
    import concourse.mybir as mybir
    import concourse.tile as tile
    from concourse import bacc
    from concourse.masks import make_identity
    from concourse.tile_rust import add_dep_helper

    F32 = mybir.dt.float32
    BF16 = mybir.dt.bfloat16
    FP8 = mybir.dt.float8e4
    AF = mybir.ActivationFunctionType
    OP = mybir.AluOpType
    DR = mybir.MatmulPerfMode.DoubleRow

    nc = bacc.Bacc("TRN2", target_bir_lowering=False, debug=False,
                   num_devices=NCORES)

    xb = nc.dram_tensor("xb", [C, HW], BF16, kind="ExternalInput")
    wq_d = nc.dram_tensor("wq", [C, C], F32, kind="ExternalInput")
    wk_d = nc.dram_tensor("wk", [C, C], F32, kind="ExternalInput")
    wv_d = nc.dram_tensor("wv", [C, C], F32, kind="ExternalInput")
    wp_d = nc.dram_tensor("wp", [C, C], F32, kind="ExternalInput")
    cst_d = nc.dram_tensor("consts", [P, 24], F32, kind="ExternalInput")
    on8_d = nc.dram_tensor("ones8", [P, 2, P], FP8, kind="ExternalInput")
    y = nc.dram_tensor("y", [C, NQ], F32, kind="ExternalOutput")

    xb_t = xb.rearrange("(cc p) n -> p cc n", p=P)
    y_t = y.rearrange("(cc p) n -> p cc n", p=P)

    with tile.TileContext(nc) as tc:
        with (
            tc.tile_pool(name="persist", bufs=1) as pers,
            tc.tile_pool(name="wnat", bufs=2) as wnp,
            tc.tile_pool(name="tmp", bufs=3) as tmp,
            tc.tile_pool(name="pt8", bufs=8) as ptp,
            tc.tile_pool(name="rdb", bufs=2) as rdp,
            tc.tile_pool(name="attn", bufs=2) as atp,
            tc.tile_pool(name="outs", bufs=3) as otp,
        ):
            # ---------------- DMA first ----------------
            # X rides FIRST on every queue (the GN stats chain is the
            # startup critical path and gates on the last X byte); weights
            # and small constants queue up behind it. No separate xq load:
            # the host rotates keys per-core so queries are X[:, :, :NQ].
            X = pers.tile([P, 2, HW], BF16)
            X8 = pers.tile([P, 2, HW], FP8)
            qmap = [nc.scalar, nc.scalar, nc.scalar,
                    nc.sync, nc.sync, nc.sync,
                    nc.gpsimd, nc.gpsimd]
            for s in range(8):
                sl = slice(512 * s, 512 * (s + 1))
                qmap[s].dma_start(out=X[:, :, sl], in_=xb_t[:, :, sl])
            wq_nat = wnp.tile([P, 2, C], F32, tag="wnat", name="wq_nat")
            nc.scalar.dma_start(out=wq_nat, in_=wq_d.rearrange("(oc p) c -> p oc c", p=P))
            wk_nat = wnp.tile([P, 2, C], F32, tag="wnat2", name="wk_nat")
            nc.sync.dma_start(out=wk_nat, in_=wk_d.rearrange("(oc p) c -> p oc c", p=P))
            wv_nat = wnp.tile([P, 2, C], F32, tag="wnat", name="wv_nat")
            nc.gpsimd.dma_start(out=wv_nat, in_=wv_d.rearrange("(oc p) c -> p oc c", p=P))
            wp_nat = wnp.tile([P, 2, C], F32, tag="wnat2", name="wp_nat")
            nc.gpsimd.dma_start(out=wp_nat, in_=wp_d.rearrange("(oc p) c -> p oc c", p=P))
            cst = pers.tile([P, 24], F32)
            nc.gpsimd.dma_start(out=cst, in_=cst_d[:, :])
            on8 = pers.tile([P, 2, P], FP8)
            nc.gpsimd.dma_start(out=on8, in_=on8_d[:, :, :])
            gm = cst[:, 0:2]
            bt = cst[:, 2:4]
            bv_t = cst[:, 4:6]
            bp_t = cst[:, 6:8]
            ind = cst[:, 8:24].rearrange("p (cc g) -> p cc g", cc=2)

            # ---------------- constant/setup tiles ----------------
            ident = pers.tile([P, P], F32)
            make_identity(nc, ident)
            shift_t = pers.tile([P, 1], F32)
            nc.vector.memset(shift_t, SHIFT)
            zero_t = pers.tile([P, 1], F32)
            nc.vector.memset(zero_t, 0.0)

            # per-chunk work (raw fp8 cast on act, GN stats on DVE) is
            # emitted in LANDING order so neither engine head-of-line blocks
            # on a chunk that arrives later than its neighbors.
            land_order = [0, 3, 6, 1, 4, 7, 2, 5]
            for s in land_order:
                sl = slice(512 * s, 512 * (s + 1))
                nc.scalar.activation(out=X8[:, :, sl], in_=X[:, :, sl],
                                     func=AF.Identity, bias=0.0)

            # ---------------- prep matmuls (PE, during DMA) ----------
            with tc.tile_pool(name="ps_prep", bufs=1, space="PSUM") as psp, \
                 tc.tile_pool(name="ps_tr", bufs=2, space="PSUM") as pst:
                # M2T[c',c] = sum_o wk[o,c'] wq[o,c]
                M2T32 = pers.tile([P, 2, C], F32)
                for cp in range(2):
                    m2ps = pst.tile([P, C], F32, tag="tr", name=f"m2ps{cp}")
                    for oc in range(2):
                        nc.tensor.matmul(m2ps, wk_nat[:, oc, cp * P:(cp + 1) * P],
                                         wq_nat[:, oc, :],
                                         start=(oc == 0), stop=(oc == 1))
                    nc.scalar.activation(out=M2T32[:, cp, :], in_=m2ps,
                                         func=AF.Identity, bias=0.0)
                # indT = 32 * ind^T, via PE transpose
                indT = pers.tile([G, 2, P], F32)
                for cc in range(2):
                    it_ps = pst.tile([G, P], F32, tag="tr2", name=f"it_ps{cc}")
                    nc.tensor.transpose(it_ps, ind[:, cc, :], ident)
                    nc.scalar.mul(out=indT[:, cc, :], in_=it_ps, mul=32.0)
                # wvT / wpT via PE transpose
                wvT32 = pers.tile([P, 2, C], F32)
                wpT32 = pers.tile([P, 2, C], F32)
                for (nat, t32) in ((wv_nat, wvT32), (wp_nat, wpT32)):
                    for rc in range(2):
                        for cc in range(2):
                            ps_t = pst.tile([P, P], F32, tag="tr2")
                            nc.tensor.transpose(
                                ps_t, nat[:, rc, cc * P:(cc + 1) * P], ident)
                            # psum->sbuf copies on act: the DVE's serial GN
                            # stats chain is the startup critical path
                            nc.scalar.activation(
                                out=t32[:, cc, rc * P:(rc + 1) * P], in_=ps_t,
                                func=AF.Identity, bias=0.0)

                # ---------------- GroupNorm stats -> A, B ----------------
                gst = psp.tile([G, 2], F32)  # per-group E[x], E[x^2]
                subs = [tmp.tile([P, 8, 6], F32, tag=f"bnsub{cc}",
                                 name=f"bnsub{cc}") for cc in range(2)]
                stat_is = []
                for s in [0, 3, 6, 1, 4, 7, 2, 5]:  # DMA landing order
                    for cc in range(2):
                        stat_is.append(nc.vector.bn_stats(
                            out=subs[cc][:, s, :],
                            in_=X[:, cc, 512 * s:512 * (s + 1)]))
                aggr_instrs = []
                for cc in range(2):
                    mv = tmp.tile([P, 2], F32, tag="mv")
                    aggr_instrs.append(nc.vector.bn_aggr(out=mv, in_=subs[cc]))
                    st2 = tmp.tile([P, 2], F32, tag="st2")
                    nc.vector.tensor_copy(st2[:, 0:1], mv[:, 0:1])
                    nc.vector.tensor_mul(st2[:, 1:2], mv[:, 0:1], mv[:, 0:1])
                    nc.vector.tensor_add(st2[:, 1:2], st2[:, 1:2], mv[:, 1:2])
                    nc.tensor.matmul(gst, ind[:, cc, :], st2,
                                     start=(cc == 0), stop=(cc == 1))
                gss = pers.tile([G, 2], F32)
                nc.vector.tensor_copy(gss, gst)
                varg = pers.tile([G, 1], F32)
                nc.vector.tensor_mul(varg, gss[:, 0:1], gss[:, 0:1])
                nc.vector.tensor_tensor(varg, gss[:, 1:2], varg, OP.subtract)
                eps_t = pers.tile([G, 1], F32)
                nc.vector.memset(eps_t, EPS)
                sdg = pers.tile([G, 1], F32)
                nc.scalar.activation(out=sdg, in_=varg, func=AF.Sqrt, bias=eps_t)
                rstdg = pers.tile([G, 1], F32)
                nc.vector.reciprocal(rstdg, sdg)
                gsb = pers.tile([G, 2], F32)
                nc.vector.tensor_copy(gsb[:, 0:1], gss[:, 0:1])
                nc.vector.tensor_copy(gsb[:, 1:2], rstdg)

                A = pers.tile([P, 2], F32)
                Bv = pers.tile([P, 2], F32)
                a_instrs = []
                for cc in range(2):
                    bc = psp.tile([P, 2], F32, tag="bc", name=f"bc{cc}")
                    nc.tensor.matmul(bc, indT[:, cc, :], gsb, start=True, stop=True)
                    a_instrs.append(nc.vector.tensor_mul(
                        A[:, cc:cc + 1], bc[:, 1:2], gm[:, cc:cc + 1]))
                    nc.vector.tensor_mul(Bv[:, cc:cc + 1], bc[:, 0:1], A[:, cc:cc + 1])
                    nc.vector.tensor_tensor(Bv[:, cc:cc + 1], bt[:, cc:cc + 1],
                                            Bv[:, cc:cc + 1], OP.subtract)

                # fold A into the fp8 weight tiles (contraction-side A
                # multiplies before quantization, x itself stays raw fp8)
                M2Tf8 = pers.tile([P, 2, C], FP8)
                wvTf8 = pers.tile([P, 2, C], FP8)
                wpT8 = pers.tile([P, 2, C], FP8)
                for cc in range(2):
                    nc.vector.tensor_scalar_mul(wvTf8[:, cc, :], wvT32[:, cc, :],
                                                A[:, cc:cc + 1])
                    nc.vector.tensor_scalar_mul(M2Tf8[:, cc, :], M2T32[:, cc, :],
                                                A[:, cc:cc + 1])
                    nc.vector.tensor_copy(wpT8[:, cc, :], wpT32[:, cc, :])

                # deferred biases: cbv = wv@B + bv ; cbp = wp@cbv + bp. The
                # PE is idle here anyway (waiting on the fp8 weight folds),
                # so the DVE round-trips cost nothing.
                cbv = pers.tile([P, 2], F32)
                cbp = pers.tile([P, 2], F32)
                for oc in range(2):
                    cb_ps = psp.tile([P, 1], F32, tag="cb", name=f"cbv_ps{oc}")
                    for cc in range(2):
                        nc.tensor.matmul(cb_ps, wvT32[:, cc, oc * P:(oc + 1) * P],
                                         Bv[:, cc:cc + 1],
                                         start=(cc == 0), stop=(cc == 1))
                    nc.vector.tensor_add(cbv[:, oc:oc + 1], cb_ps,
                                         bv_t[:, oc:oc + 1])
                for oc in range(2):
                    cb_ps2 = psp.tile([P, 1], F32, tag="cb2", name=f"cbp_ps{oc}")
                    for cc in range(2):
                        nc.tensor.matmul(cb_ps2, wpT32[:, cc, oc * P:(oc + 1) * P],
                                         cbv[:, cc:cc + 1],
                                         start=(cc == 0), stop=(cc == 1))
                    nc.vector.tensor_add(cbp[:, oc:oc + 1], cb_ps2,
                                         bp_t[:, oc:oc + 1])

            # fp8 queries (per-block chunks; block 0 first so attention can
            # start) and the bias-carrying residual copy.
            Xq8 = pers.tile([P, 2, NQ], FP8)
            Xq32r = pers.tile([P, 2, NQ], F32)

            def emit_xq8(qb):
                qs = slice(QB * qb, QB * (qb + 1))
                for cc in range(2):
                    nc.vector.tensor_scalar(out=Xq8[:, cc, qs],
                                            in0=X[:, cc, qs],
                                            scalar1=A[:, cc:cc + 1],
                                            scalar2=Bv[:, cc:cc + 1],
                                            op0=OP.mult, op1=OP.add)

            def emit_xq32r(qb):
                qs = slice(QB * qb, QB * (qb + 1))
                for cc in range(2):
                    nc.vector.tensor_scalar(out=Xq32r[:, cc, qs],
                                            in0=X[:, cc, qs],
                                            scalar1=cbp[:, cc:cc + 1],
                                            scalar2=None,
                                            op0=OP.add)

            # block 0's query chunk goes through the act engine (idle here;
            # the DVE is the startup-critical resource): Identity with
            # per-partition scale=A, bias=B
            for cc in range(2):
                nc.scalar.activation(out=Xq8[:, cc, 0:QB], in_=X[:, cc, 0:QB],
                                     func=AF.Identity,
                                     scale=A[:, cc:cc + 1],
                                     bias=Bv[:, cc:cc + 1])

            # ---------------- VT phase: VT8[k, c] = (x^T (wvT.A)) in fp8 ----
            # Two key chunks share one psum bank (a matmul with start=True
            # zeroes the whole 2KB zero-region, so the second chunk's pair
            # accumulates onto zeros with start=False); one [128,512] cast
            # per pair halves the per-instruction cast overhead.
            VT8 = pers.tile([P, NMB, C], FP8)
            with tc.tile_pool(name="ps_vt", bufs=1, space="PSUM") as psv:
                # PE p-state warm-up: the tensor engine has been idle through
                # the DMA/GN window and would run the whole VT stream at the
                # mid p-state. Burn ~3us of garbage fp8 matmuls, gated on A
                # so they fill exactly the fold-chain window before VT.
                # stage 1 fires once ~10 stats chunks are done (the PE has
                # been idle since the prep matmuls); stage 2 bridges the
                # A-fold window so the clock carries into the VT stream.
                warm = psv.tile([P, QB], F32, tag="warm")
                for w in range(8):
                    wi = nc.tensor.matmul(warm, X8[:, :, 0:P],
                                          X8[:, :, 0:QB],
                                          start=True, stop=True, perf_mode=DR,
                                          skip_group_check=True)
                    if w == 0:
                        add_dep_helper(wi.ins, stat_is[9].ins, True,
                                       "pe warmup stage 1")
                for w in range(5):
                    wi = nc.tensor.matmul(warm, X8[:, :, 0:P],
                                          X8[:, :, 0:QB],
                                          start=True, stop=True, perf_mode=DR,
                                          skip_group_check=True)
                    if w == 0:
                        for ai in a_instrs:
                            add_dep_helper(wi.ins, ai.ins, True,
                                           "pe warmup stage 2")
                for t in range(NMB // 2):
                    vt_ps = psv.tile([P, 2, C], F32, tag="vt", bufs=5)
                    for i in range(2):
                        m = 2 * t + i
                        nc.tensor.matmul(vt_ps[:, i, :],
                                         X8[:, :, P * m:P * (m + 1)],
                                         wvTf8,
                                         start=(i == 0), stop=True,
                                         skip_group_check=(i == 1),
                                         perf_mode=DR)
                    if t % 2 == 0:
                        nc.scalar.activation(out=VT8[:, 2 * t:2 * t + 2, :],
                                             in_=vt_ps, func=AF.Identity,
                                             bias=0.0)
                    else:
                        nc.vector.tensor_copy(VT8[:, 2 * t:2 * t + 2, :], vt_ps)

            # ---------------- KS phase: KS8 = (M2T.A) @ x in fp8 ----------
            # ---------------- fp8 attention ----------------
            # Per pair j (256 keys): S = two DoubleRow matmuls (one per key
            # chunk, both channel halves contracted at once), one exp over
            # [128,1024] psum -> fp8, then (deferred) PV + denominator
            # DoubleRow matmuls. The act engine is the pacing resource; all
            # DVE work sits in per-block boundary slots.
            #
            # The KS production runs INSIDE the attention scope: its psum
            # pool (2 banks) coexists with s_ps (4) + pv (2), and releases
            # before the first den/proj tile commits the aux pool (2) -- an
            # exact 8-bank ledger both before and after. Block 0's first six
            # pairs interleave with the KS stream so the exp pipeline starts
            # as soon as KS mb0 is cast, not after the whole phase.
            KS8 = pers.tile([P, 2, HW], FP8)
            with (
                tc.tile_pool(name="ps_s", bufs=2, space="PSUM") as pss,
                tc.tile_pool(name="ps_pv", bufs=2, space="PSUM") as pspv,
            ):
                pools = {}
                pvs = {}     # qb -> (pv0, pv1)
                dens = {}    # qb -> den psum
                rdbs = {}    # qb -> 32/den
                attns = {}   # qb -> fp8 normalized attention
                pts = {}     # (qb, j) -> pT8 tile

                def emit_s_exp(qb, j):
                    qs = slice(QB * qb, QB * (qb + 1))
                    s_ps = pss.tile([P, 2, QB], F32, tag="s", name=f"s{qb}_{j}")
                    for i in range(2):
                        m = 2 * j + i
                        nc.tensor.matmul(s_ps[:, i, :],
                                         KS8[:, :, P * m:P * (m + 1)],
                                         Xq8[:, :, qs],
                                         start=True, stop=True, perf_mode=DR)
                    pt = ptp.tile([P, 2, QB], FP8, tag="pt", name=f"pt{qb}_{j}")
                    nc.scalar.activation(out=pt, in_=s_ps, func=AF.Exp,
                                         scale=0.0625, bias=shift_t)
                    pts[(qb, j)] = pt

                def emit_pv(qb, j):
                    if j == 0:
                        pvs[qb] = (
                            pspv.tile([P, QB], F32, tag="pv", name=f"pv0_{qb}"),
                            pspv.tile([P, QB], F32, tag="pv", name=f"pv1_{qb}"),
                        )
                    pv0, pv1 = pvs[qb]
                    pt = pts[(qb, j)]
                    nc.tensor.matmul(pv0, VT8[:, 2 * j:2 * j + 2, 0:P], pt,
                                     start=(j == 0), stop=(j == NPAIR - 1),
                                     perf_mode=DR)
                    nc.tensor.matmul(pv1, VT8[:, 2 * j:2 * j + 2, P:C], pt,
                                     start=(j == 0), stop=(j == NPAIR - 1),
                                     perf_mode=DR)

                def emit_den(qb, j):
                    if j == 0:
                        dens[qb] = pools["aux"].tile([P, QB], F32, tag="aux",
                                                     name=f"den_{qb}")
                    nc.tensor.matmul(dens[qb], on8, pts[(qb, j)],
                                     start=(j == 0), stop=(j == NPAIR - 1),
                                     perf_mode=DR)

                def emit_recip(qb):
                    # ~18-bit approx: den is a well-conditioned positive sum,
                    # and a 4e-6 relative error on the softmax denominator is
                    # invisible next to the fp8 quantization noise.
                    rdb = rdp.tile([P, QB], F32, tag="rdb", name=f"rdb_{qb}")
                    nc.vector.reciprocal_approx_fast(rdb, dens[qb])
                    rdbs[qb] = rdb

                def emit_norm(qb):
                    at = atp.tile([P, 2, QB], FP8, tag="attn", name=f"at_{qb}")
                    pv0, pv1 = pvs[qb]
                    nc.vector.tensor_mul(at[:, 0, :], pv0, rdbs[qb])
                    nc.vector.tensor_mul(at[:, 1, :], pv1, rdbs[qb])
                    attns[qb] = at

                def emit_proj(qb, oc):
                    # output DMAs stay off the act-engine queue: a waiting
                    # descriptor competes with the exp stream's sequencer.
                    qs = slice(QB * qb, QB * (qb + 1))
                    po = pools["aux"].tile([P, QB], F32, tag="aux",
                                           name=f"po{qb}_{oc}")
                    nc.tensor.matmul(po, wpT8[:, :, oc * P:(oc + 1) * P],
                                     attns[qb], start=True, stop=True,
                                     perf_mode=DR)
                    outsb = otp.tile([P, QB], F32, tag="outsb")
                    nc.vector.scalar_tensor_tensor(
                        out=outsb, in0=po, scalar=1.0 / 32.0,
                        in1=Xq32r[:, oc, qs], op0=OP.mult, op1=OP.add)
                    # all stores on the sync queue: it drains instantly at
                    # kernel end, while a gpsimd-queue DMA in flight charges
                    # ~2.5us to the final drain
                    nc.sync.dma_start(out=y_t[:, oc, qs], in_=outsb)

                # KS production (fp8 DoubleRow from X8) interleaved with
                # block 0's first pairs; mb0's casts ride the act engine so
                # the exp stream starts immediately behind them.
                with tc.tile_pool(name="ps_ks", bufs=2, space="PSUM") as psk:
                    def emit_ks(mb):
                        for co in range(2):
                            ks_ps = psk.tile([P, QB], F32, tag="ks")
                            nc.tensor.matmul(
                                ks_ps, M2Tf8[:, :, co * P:(co + 1) * P],
                                X8[:, :, QB * mb:QB * (mb + 1)],
                                start=True, stop=True, perf_mode=DR)
                            ksl = KS8[:, co, QB * mb:QB * (mb + 1)]
                            if mb < 2:
                                nc.scalar.activation(out=ksl, in_=ks_ps,
                                                     func=AF.Identity, bias=0.0)
                            else:
                                nc.vector.tensor_copy(ksl, ks_ps)

                    emit_ks(0)
                    emit_ks(1)
                    for p in range(6):
                        emit_ks(p + 2)
                        emit_s_exp(0, p)
                        if p >= 2:
                            emit_pv(0, p - 2)

                # schedule: exp stream never waits; PE work (PV/den) and all
                # DVE/projection work for block qb-1 hide inside block qb.
                with tc.tile_pool(name="ps_aux", bufs=2, space="PSUM") as psx:
                    pools["aux"] = psx
                    for qb in range(NQB):
                        for j in range(NPAIR):
                            if qb == 0 and j < 6:
                                continue  # emitted inside the KS scope
                            emit_s_exp(qb, j)
                            prev = qb - 1
                            if prev >= 0:
                                if j == 0:
                                    emit_pv(prev, NPAIR - 2)
                                    emit_den(prev, NPAIR - 2)
                                elif j == 1:
                                    emit_pv(prev, NPAIR - 1)
                                    emit_den(prev, NPAIR - 1)
                                elif j == 2:
                                    emit_recip(prev)
                                elif j == 3:
                                    emit_norm(prev)
                                elif j == 4:
                                    emit_proj(prev, 0)
                                elif j == 5:
                                    emit_proj(prev, 1)
                            if j == 6:
                                for jj in (range(4, 5) if qb == 0 else
                                           range(5)):
                                    emit_pv(qb, jj)
                            elif j == 7:
                                emit_pv(qb, 5)
                                for jj in range(6):
                                    emit_den(qb, jj)
                            elif 8 <= j <= 15:
                                emit_pv(qb, j - 2)
                                emit_den(qb, j - 2)
                            if j == 10 and qb + 1 < NQB:
                                emit_xq8(qb + 1)
                            elif j == 11:
                                emit_xq32r(qb)
                            if j == 15:
                                if qb == NQB - 1:
                                    # shorten the drain: pair 14's PV/den
                                    # can ride right behind its exp
                                    emit_pv(qb, NPAIR - 2)
                                    emit_den(qb, NPAIR - 2)

                    # drain: last block's tail in 256-wide slices so the
                    # normalize/project/store pipeline overlaps itself
                    qb = NQB - 1
                    emit_pv(qb, NPAIR - 1)
                    emit_den(qb, NPAIR - 1)
                    emit_recip(qb)
                    at = atp.tile([P, 2, QB], FP8, tag="attn", name="at_drain")
                    pv0, pv1 = pvs[qb]
                    for hh in range(2):
                        h = slice(256 * hh, 256 * (hh + 1))
                        nc.vector.tensor_mul(at[:, 0, h], pv0[:, h],
                                             rdbs[qb][:, h])
                        nc.vector.tensor_mul(at[:, 1, h], pv1[:, h],
                                             rdbs[qb][:, h])
                        for oc in range(2):
                            po = psx.tile([P, QB], F32, tag="aux",
                                          name=f"poD{hh}_{oc}")
                            nc.tensor.matmul(po[:, 0:256],
                                             wpT8[:, :, oc * P:(oc + 1) * P],
                                             at[:, :, h], start=True,
                                             stop=True, perf_mode=DR)
                            outsb = otp.tile([P, QB], F32, tag="outsb")
                            hq = slice(QB * qb + 256 * hh,
                                       QB * qb + 256 * (hh + 1))
                            nc.vector.scalar_tensor_tensor(
                                out=outsb[:, 0:256], in0=po[:, 0:256],
                                scalar=1.0 / 32.0, in1=Xq32r[:, oc, hq],
                                op0=OP.mult, op1=OP.add)
                            # sync-queue only: the gpsimd queue's end-of-
                            # kernel drain charges ~2.5us for a DMA still in
                            # flight there
                            nc.sync.dma_start(out=y_t[:, oc, hq],
                                              in_=outsb[:, 0:256])

    nc.compile()
    return nc


def _get_nc():
    if "nc" not in _cache:
        _cache["nc"] = _build()
    return _cache["nc"]


def _in_maps(inputs):
    """Per-core input maps. Core p = 2*b + h gets batch b's image with the
    key columns rotated so its query half sits at columns [0, NQ) -- keys
    are an unordered set under attention, so the rotation is free and saves
    a separate 2MB query load."""
    import ml_dtypes
    x = np.ascontiguousarray(np.asarray(inputs["x"], dtype=np.float32)
                             ).reshape(4, C, HW)
    common = {
        "consts": _pack_consts(inputs["gn_gamma"], inputs["gn_beta"],
                               inputs["bv"], inputs["bp"]),
        "ones8": _ones8(),
        "wq": np.asarray(inputs["wq"], np.float32),
        "wk": np.asarray(inputs["wk"], np.float32),
        "wv": np.asarray(inputs["wv"], np.float32),
        "wp": np.asarray(inputs["wp"], np.float32),
    }
    in_maps = []
    xb16 = [x[b].astype(ml_dtypes.bfloat16) for b in range(4)]
    for p in range(NCORES):
        b, h = divmod(p, 2)
        m = dict(common)
        m["xb"] = (xb16[b] if h == 0 else
                   np.ascontiguousarray(np.roll(xb16[b], -NQ, axis=1)))
        in_maps.append(m)
    return in_maps


def kernel(**inputs):
    from concourse.bass_utils import run_bass_kernel_spmd

    nc = _get_nc()
    res = run_bass_kernel_spmd(nc, _in_maps(inputs), list(range(NCORES)))
    out = np.empty((4, C, HW), np.float32)
    for p in range(NCORES):
        b, h = divmod(p, 2)
        out[b, :, h * NQ:(h + 1) * NQ] = res.results[p]["y"]
    return out.reshape(4, C, 64, 64)


# revision 75
# speedup vs baseline: 1.1712x; 1.1712x over previous
"""Trainium2 Bass kernel for nn_AttentionBlock (GroupNorm + single-head attention + residual).

Reference computation (b=4, c=256, h=w=64, n=h*w=4096):
    xn = GroupNorm(x, groups=8) * gamma + beta          # [b,c,n]
    q/k/v = w{q,k,v} @ xn + b{q,k,v}                    # 1x1 conv = channel matmul
    S = (q^T k) / sqrt(c);  P = softmax(S, axis=-1)     # [b,n,n]
    out = wp @ (v @ P^T) + bp + x

Sharding: pure data parallel, no collectives. Core p = 2*b + h handles batch b
and query half h (2048 queries), computing GroupNorm stats + keys/values for
its batch redundantly with its pair core. Each core returns y = out[b][:, half].

Math restructure:
  - GN fold: xn = A*x + B per channel (A = rstd*gamma, B = beta - mean*A).
  - S = xn_q^T M2 xn_k with M2 = wq^T wk. Key-side additive constants drop
    out of the softmax exactly (uniform per-query shift); bq == 0.
    KS = (M2^T . A) @ x, so S = KS^T xn_q -- no Q needed.
  - The four big attention matmuls (S, PV, softmax-denominator, projection)
    run in fp8e4m3 with MatmulPerfMode.DoubleRow: 2 fp8 rows/cycle and both
    128-deep contraction halves in one instruction -> 4x the fp32r rate.
    exp(s/16 - 3) keeps P inside fp8 range (max |s|/16 ~ 7.6; e^4.6=101 <
    240); the -3 shift cancels in the softmax. fp8 quantization noise lands
    on the attention branch only (~2.6% of the output after the residual):
    measured end-to-end rel err ~5e-3 vs the 2e-2 gate.
  - Softmax denominator on the PE: a DoubleRow matmul with an all-(1/32)
    fp8 stationary both reduces over keys and broadcasts den/32 across all
    128 partitions; a full-width [128,512] DVE reciprocal then gives
    32/den, so the normalize multiply feeds the fp8 projection at a good
    scale (attn*32 ~ N(0,0.83^2)) and a single scalar_tensor_tensor fuses
    the /32, deferred bias (pre-added into the residual) and residual add.
  - v bias deferral: cbv = wv@B + bv is pushed through the projection into
    cbp = wp@cbv + bp, which is pre-added into the residual copy.
"""

import numpy as np

P = 128
C = 256
HW = 4096
NQ = 2048
G = 8
EPS = 1e-5
NCORES = 8
QB = 512           # query block
NMB = HW // P      # 32 key chunks of 128
NPAIR = NMB // 2   # 16 key-chunk pairs per query block
NQB = NQ // QB     # 4 query blocks
SHIFT = -3.0       # exp bias: cancels in softmax, keeps P in fp8e4m3 range

_cache = {}


def _pack_consts(gamma, beta, bv, bp):
    """One packed [128, 24] tile: gamma/beta/bv/bp (chunked by 128) and the
    group-indicator matrix (value 1/32, block-diagonal over 32-channel groups)."""
    cst = np.zeros((P, 24), np.float32)
    for i, v in enumerate((gamma, beta, bv, bp)):
        cst[:, 2 * i:2 * i + 2] = np.asarray(v, np.float32).reshape(2, P).T
    for cc in range(2):
        for j in range(4):
            cst[32 * j:32 * (j + 1), 8 + cc * G + 4 * cc + j] = 1.0 / 32.0
    return cst


def _ones8():
    import ml_dtypes
    return np.full((P, 2, P), 1.0 / 32.0, ml_dtypes.float8_e4m3)


def _build():
    import concourse.bass as bass
    import concourse.mybir as mybir
    import concourse.tile as tile
    from concourse import bacc
    from concourse.masks import make_identity
    from concourse.tile_rust import add_dep_helper

    F32 = mybir.dt.float32
    BF16 = mybir.dt.bfloat16
    FP8 = mybir.dt.float8e4
    AF = mybir.ActivationFunctionType
    OP = mybir.AluOpType
    DR = mybir.MatmulPerfMode.DoubleRow

    nc = bacc.Bacc("TRN2", target_bir_lowering=False, debug=False,
                   num_devices=NCORES)

    xb = nc.dram_tensor("xb", [C, HW], BF16, kind="ExternalInput")
    wq_d = nc.dram_tensor("wq", [C, C], F32, kind="ExternalInput")
    wk_d = nc.dram_tensor("wk", [C, C], F32, kind="ExternalInput")
    wv_d = nc.dram_tensor("wv", [C, C], F32, kind="ExternalInput")
    wp_d = nc.dram_tensor("wp", [C, C], F32, kind="ExternalInput")
    cst_d = nc.dram_tensor("consts", [P, 24], F32, kind="ExternalInput")
    on8_d = nc.dram_tensor("ones8", [P, 2, P], FP8, kind="ExternalInput")
    y = nc.dram_tensor("y", [C, NQ], F32, kind="ExternalOutput")

    xb_t = xb.rearrange("(cc p) n -> p cc n", p=P)
    y_t = y.rearrange("(cc p) n -> p cc n", p=P)

    with tile.TileContext(nc) as tc:
        with (
            tc.tile_pool(name="persist", bufs=1) as pers,
            tc.tile_pool(name="wnat", bufs=2) as wnp,
            tc.tile_pool(name="tmp", bufs=3) as tmp,
            tc.tile_pool(name="pt8", bufs=8) as ptp,
            tc.tile_pool(name="rdb", bufs=2) as rdp,
            tc.tile_pool(name="attn", bufs=2) as atp,
            tc.tile_pool(name="outs", bufs=3) as otp,
        ):
            # ---------------- DMA first ----------------
            # X rides FIRST on every queue (the GN stats chain is the
            # startup critical path and gates on the last X byte); weights
            # and small constants queue up behind it. No separate xq load:
            # the host rotates keys per-core so queries are X[:, :, :NQ].
            X = pers.tile([P, 2, HW], BF16)
            X8 = pers.tile([P, 2, HW], FP8)
            qmap = [nc.scalar, nc.scalar, nc.scalar,
                    nc.sync, nc.sync, nc.sync,
                    nc.gpsimd, nc.gpsimd]
            for s in range(8):
                sl = slice(512 * s, 512 * (s + 1))
                qmap[s].dma_start(out=X[:, :, sl], in_=xb_t[:, :, sl])
            wq_nat = wnp.tile([P, 2, C], F32, tag="wnat", name="wq_nat")
            nc.scalar.dma_start(out=wq_nat, in_=wq_d.rearrange("(oc p) c -> p oc c", p=P))
            wk_nat = wnp.tile([P, 2, C], F32, tag="wnat2", name="wk_nat")
            nc.sync.dma_start(out=wk_nat, in_=wk_d.rearrange("(oc p) c -> p oc c", p=P))
            wv_nat = wnp.tile([P, 2, C], F32, tag="wnat", name="wv_nat")
            nc.gpsimd.dma_start(out=wv_nat, in_=wv_d.rearrange("(oc p) c -> p oc c", p=P))
            wp_nat = wnp.tile([P, 2, C], F32, tag="wnat2", name="wp_nat")
            nc.gpsimd.dma_start(out=wp_nat, in_=wp_d.rearrange("(oc p) c -> p oc c", p=P))
            cst = pers.tile([P, 24], F32)
            nc.gpsimd.dma_start(out=cst, in_=cst_d[:, :])
            on8 = pers.tile([P, 2, P], FP8)
            nc.gpsimd.dma_start(out=on8, in_=on8_d[:, :, :])
            gm = cst[:, 0:2]
            bt = cst[:, 2:4]
            bv_t = cst[:, 4:6]
            bp_t = cst[:, 6:8]
            ind = cst[:, 8:24].rearrange("p (cc g) -> p cc g", cc=2)

            # ---------------- constant/setup tiles ----------------
            ident = pers.tile([P, P], F32)
            make_identity(nc, ident)
            shift_t = pers.tile([P, 1], F32)
            nc.vector.memset(shift_t, SHIFT)
            zero_t = pers.tile([P, 1], F32)
            nc.vector.memset(zero_t, 0.0)

            # per-chunk work (raw fp8 cast on act, GN stats on DVE) is
            # emitted in LANDING order so neither engine head-of-line blocks
            # on a chunk that arrives later than its neighbors.
            land_order = [0, 3, 6, 1, 4, 7, 2, 5]
            for s in land_order:
                sl = slice(512 * s, 512 * (s + 1))
                nc.scalar.activation(out=X8[:, :, sl], in_=X[:, :, sl],
                                     func=AF.Identity, bias=0.0)

            # ---------------- prep matmuls (PE, during DMA) ----------
            with tc.tile_pool(name="ps_prep", bufs=1, space="PSUM") as psp, \
                 tc.tile_pool(name="ps_tr", bufs=2, space="PSUM") as pst:
                # M2T[c',c] = sum_o wk[o,c'] wq[o,c]
                M2T32 = pers.tile([P, 2, C], F32)
                for cp in range(2):
                    m2ps = pst.tile([P, C], F32, tag="tr", name=f"m2ps{cp}")
                    for oc in range(2):
                        nc.tensor.matmul(m2ps, wk_nat[:, oc, cp * P:(cp + 1) * P],
                                         wq_nat[:, oc, :],
                                         start=(oc == 0), stop=(oc == 1))
                    nc.scalar.activation(out=M2T32[:, cp, :], in_=m2ps,
                                         func=AF.Identity, bias=0.0)
                # indT = 32 * ind^T, via PE transpose
                indT = pers.tile([G, 2, P], F32)
                for cc in range(2):
                    it_ps = pst.tile([G, P], F32, tag="tr2", name=f"it_ps{cc}")
                    nc.tensor.transpose(it_ps, ind[:, cc, :], ident)
                    nc.scalar.mul(out=indT[:, cc, :], in_=it_ps, mul=32.0)
                # wvT / wpT via PE transpose
                wvT32 = pers.tile([P, 2, C], F32)
                wpT32 = pers.tile([P, 2, C], F32)
                for (nat, t32) in ((wv_nat, wvT32), (wp_nat, wpT32)):
                    for rc in range(2):
                        for cc in range(2):
                            ps_t = pst.tile([P, P], F32, tag="tr2")
                            nc.tensor.transpose(
                                ps_t, nat[:, rc, cc * P:(cc + 1) * P], ident)
                            # psum->sbuf copies on act: the DVE's serial GN
                            # stats chain is the startup critical path
                            nc.scalar.activation(
                                out=t32[:, cc, rc * P:(rc + 1) * P], in_=ps_t,
                                func=AF.Identity, bias=0.0)

                # ---------------- GroupNorm stats -> A, B ----------------
                gst = psp.tile([G, 2], F32)  # per-group E[x], E[x^2]
                subs = [tmp.tile([P, 8, 6], F32, tag=f"bnsub{cc}",
                                 name=f"bnsub{cc}") for cc in range(2)]
                stat_is = []
                for s in [0, 3, 6, 1, 4, 7, 2, 5]:  # DMA landing order
                    for cc in range(2):
                        stat_is.append(nc.vector.bn_stats(
                            out=subs[cc][:, s, :],
                            in_=X[:, cc, 512 * s:512 * (s + 1)]))
                aggr_instrs = []
                for cc in range(2):
                    mv = tmp.tile([P, 2], F32, tag="mv")
                    aggr_instrs.append(nc.vector.bn_aggr(out=mv, in_=subs[cc]))
                    st2 = tmp.tile([P, 2], F32, tag="st2")
                    nc.vector.tensor_copy(st2[:, 0:1], mv[:, 0:1])
                    nc.vector.tensor_mul(st2[:, 1:2], mv[:, 0:1], mv[:, 0:1])
                    nc.vector.tensor_add(st2[:, 1:2], st2[:, 1:2], mv[:, 1:2])
                    nc.tensor.matmul(gst, ind[:, cc, :], st2,
                                     start=(cc == 0), stop=(cc == 1))
                gss = pers.tile([G, 2], F32)
                nc.vector.tensor_copy(gss, gst)
                varg = pers.tile([G, 1], F32)
                nc.vector.tensor_mul(varg, gss[:, 0:1], gss[:, 0:1])
                nc.vector.tensor_tensor(varg, gss[:, 1:2], varg, OP.subtract)
                eps_t = pers.tile([G, 1], F32)
                nc.vector.memset(eps_t, EPS)
                sdg = pers.tile([G, 1], F32)
                nc.scalar.activation(out=sdg, in_=varg, func=AF.Sqrt, bias=eps_t)
                rstdg = pers.tile([G, 1], F32)
                nc.vector.reciprocal(rstdg, sdg)
                gsb = pers.tile([G, 2], F32)
                nc.vector.tensor_copy(gsb[:, 0:1], gss[:, 0:1])
                nc.vector.tensor_copy(gsb[:, 1:2], rstdg)

                A = pers.tile([P, 2], F32)
                Bv = pers.tile([P, 2], F32)
                a_instrs = []
                for cc in range(2):
                    bc = psp.tile([P, 2], F32, tag="bc", name=f"bc{cc}")
                    nc.tensor.matmul(bc, indT[:, cc, :], gsb, start=True, stop=True)
                    a_instrs.append(nc.vector.tensor_mul(
                        A[:, cc:cc + 1], bc[:, 1:2], gm[:, cc:cc + 1]))
                    nc.vector.tensor_mul(Bv[:, cc:cc + 1], bc[:, 0:1], A[:, cc:cc + 1])
                    nc.vector.tensor_tensor(Bv[:, cc:cc + 1], bt[:, cc:cc + 1],
                                            Bv[:, cc:cc + 1], OP.subtract)

                # fold A into the fp8 weight tiles (contraction-side A
                # multiplies before quantization, x itself stays raw fp8)
                M2Tf8 = pers.tile([P, 2, C], FP8)
                wvTf8 = pers.tile([P, 2, C], FP8)
                wpT8 = pers.tile([P, 2, C], FP8)
                for cc in range(2):
                    nc.vector.tensor_scalar_mul(wvTf8[:, cc, :], wvT32[:, cc, :],
                                                A[:, cc:cc + 1])
                    nc.vector.tensor_scalar_mul(M2Tf8[:, cc, :], M2T32[:, cc, :],
                                                A[:, cc:cc + 1])
                    nc.vector.tensor_copy(wpT8[:, cc, :], wpT32[:, cc, :])

                # deferred biases: cbv = wv@B + bv ; cbp = wp@cbv + bp. The
                # PE is idle here anyway (waiting on the fp8 weight folds),
                # so the DVE round-trips cost nothing.
                cbv = pers.tile([P, 2], F32)
                cbp = pers.tile([P, 2], F32)
                for oc in range(2):
                    cb_ps = psp.tile([P, 1], F32, tag="cb", name=f"cbv_ps{oc}")
                    for cc in range(2):
                        nc.tensor.matmul(cb_ps, wvT32[:, cc, oc * P:(oc + 1) * P],
                                         Bv[:, cc:cc + 1],
                                         start=(cc == 0), stop=(cc == 1))
                    nc.vector.tensor_add(cbv[:, oc:oc + 1], cb_ps,
                                         bv_t[:, oc:oc + 1])
                for oc in range(2):
                    cb_ps2 = psp.tile([P, 1], F32, tag="cb2", name=f"cbp_ps{oc}")
                    for cc in range(2):
                        nc.tensor.matmul(cb_ps2, wpT32[:, cc, oc * P:(oc + 1) * P],
                                         cbv[:, cc:cc + 1],
                                         start=(cc == 0), stop=(cc == 1))
                    nc.vector.tensor_add(cbp[:, oc:oc + 1], cb_ps2,
                                         bp_t[:, oc:oc + 1])

            # fp8 queries (per-block chunks; block 0 first so attention can
            # start) and the bias-carrying residual copy.
            Xq8 = pers.tile([P, 2, NQ], FP8)
            Xq32r = pers.tile([P, 2, NQ], F32)

            def emit_xq8(qb):
                qs = slice(QB * qb, QB * (qb + 1))
                for cc in range(2):
                    nc.vector.tensor_scalar(out=Xq8[:, cc, qs],
                                            in0=X[:, cc, qs],
                                            scalar1=A[:, cc:cc + 1],
                                            scalar2=Bv[:, cc:cc + 1],
                                            op0=OP.mult, op1=OP.add)

            def emit_xq32r(qb):
                qs = slice(QB * qb, QB * (qb + 1))
                for cc in range(2):
                    nc.vector.tensor_scalar(out=Xq32r[:, cc, qs],
                                            in0=X[:, cc, qs],
                                            scalar1=cbp[:, cc:cc + 1],
                                            scalar2=None,
                                            op0=OP.add)

            # block 0's query chunk goes through the act engine (idle here;
            # the DVE is the startup-critical resource): Identity with
            # per-partition scale=A, bias=B
            for cc in range(2):
                nc.scalar.activation(out=Xq8[:, cc, 0:QB], in_=X[:, cc, 0:QB],
                                     func=AF.Identity,
                                     scale=A[:, cc:cc + 1],
                                     bias=Bv[:, cc:cc + 1])

            # ---------------- VT phase: VT8[k, c] = (x^T (wvT.A)) in fp8 ----
            # Two key chunks share one psum bank (a matmul with start=True
            # zeroes the whole 2KB zero-region, so the second chunk's pair
            # accumulates onto zeros with start=False); one [128,512] cast
            # per pair halves the per-instruction cast overhead.
            VT8 = pers.tile([P, NMB, C], FP8)
            with tc.tile_pool(name="ps_vt", bufs=1, space="PSUM") as psv:
                # PE p-state warm-up: the tensor engine has been idle through
                # the DMA/GN window and would run the whole VT stream at the
                # mid p-state. Burn ~3us of garbage fp8 matmuls, gated on A
                # so they fill exactly the fold-chain window before VT.
                # stage 1 fires once ~10 stats chunks are done (the PE has
                # been idle since the prep matmuls); stage 2 bridges the
                # A-fold window so the clock carries into the VT stream.
                warm = psv.tile([P, QB], F32, tag="warm")
                for w in range(8):
                    wi = nc.tensor.matmul(warm, X8[:, :, 0:P],
                                          X8[:, :, 0:QB],
                                          start=True, stop=True, perf_mode=DR,
                                          skip_group_check=True)
                    if w == 0:
                        add_dep_helper(wi.ins, stat_is[9].ins, True,
                                       "pe warmup stage 1")
                for w in range(5):
                    wi = nc.tensor.matmul(warm, X8[:, :, 0:P],
                                          X8[:, :, 0:QB],
                                          start=True, stop=True, perf_mode=DR,
                                          skip_group_check=True)
                    if w == 0:
                        for ai in a_instrs:
                            add_dep_helper(wi.ins, ai.ins, True,
                                           "pe warmup stage 2")
                for t in range(NMB // 2):
                    vt_ps = psv.tile([P, 2, C], F32, tag="vt", bufs=5)
                    for i in range(2):
                        m = 2 * t + i
                        nc.tensor.matmul(vt_ps[:, i, :],
                                         X8[:, :, P * m:P * (m + 1)],
                                         wvTf8,
                                         start=(i == 0), stop=True,
                                         skip_group_check=(i == 1),
                                         perf_mode=DR)
                    if t % 2 == 0:
                        nc.scalar.activation(out=VT8[:, 2 * t:2 * t + 2, :],
                                             in_=vt_ps, func=AF.Identity,
                                             bias=0.0)
                    else:
                        nc.vector.tensor_copy(VT8[:, 2 * t:2 * t + 2, :], vt_ps)

            # ---------------- KS phase: KS8 = (M2T.A) @ x in fp8 ----------
            # ---------------- fp8 attention ----------------
            # Per pair j (256 keys): S = two DoubleRow matmuls (one per key
            # chunk, both channel halves contracted at once), one exp over
            # [128,1024] psum -> fp8, then (deferred) PV + denominator
            # DoubleRow matmuls. The act engine is the pacing resource; all
            # DVE work sits in per-block boundary slots.
            #
            # The KS production runs INSIDE the attention scope: its psum
            # pool (2 banks) coexists with s_ps (4) + pv (2), and releases
            # before the first den/proj tile commits the aux pool (2) -- an
            # exact 8-bank ledger both before and after. Block 0's first six
            # pairs interleave with the KS stream so the exp pipeline starts
            # as soon as KS mb0 is cast, not after the whole phase.
            KS8 = pers.tile([P, 2, HW], FP8)
            with (
                tc.tile_pool(name="ps_s", bufs=2, space="PSUM") as pss,
                tc.tile_pool(name="ps_pv", bufs=2, space="PSUM") as pspv,
            ):
                pools = {}
                pvs = {}     # qb -> (pv0, pv1)
                dens = {}    # qb -> den psum
                rdbs = {}    # qb -> 32/den
                attns = {}   # qb -> fp8 normalized attention
                pts = {}     # (qb, j) -> pT8 tile

                def emit_s_exp(qb, j):
                    qs = slice(QB * qb, QB * (qb + 1))
                    s_ps = pss.tile([P, 2, QB], F32, tag="s", name=f"s{qb}_{j}")
                    for i in range(2):
                        m = 2 * j + i
                        nc.tensor.matmul(s_ps[:, i, :],
                                         KS8[:, :, P * m:P * (m + 1)],
                                         Xq8[:, :, qs],
                                         start=True, stop=True, perf_mode=DR)
                    pt = ptp.tile([P, 2, QB], FP8, tag="pt", name=f"pt{qb}_{j}")
                    nc.scalar.activation(out=pt, in_=s_ps, func=AF.Exp,
                                         scale=0.0625, bias=shift_t)
                    pts[(qb, j)] = pt

                def emit_pv(qb, j):
                    if j == 0:
                        pvs[qb] = (
                            pspv.tile([P, QB], F32, tag="pv", name=f"pv0_{qb}"),
                            pspv.tile([P, QB], F32, tag="pv", name=f"pv1_{qb}"),
                        )
                    pv0, pv1 = pvs[qb]
                    pt = pts[(qb, j)]
                    nc.tensor.matmul(pv0, VT8[:, 2 * j:2 * j + 2, 0:P], pt,
                                     start=(j == 0), stop=(j == NPAIR - 1),
                                     perf_mode=DR)
                    nc.tensor.matmul(pv1, VT8[:, 2 * j:2 * j + 2, P:C], pt,
                                     start=(j == 0), stop=(j == NPAIR - 1),
                                     perf_mode=DR)

                def emit_den(qb, j):
                    if j == 0:
                        dens[qb] = pools["aux"].tile([P, QB], F32, tag="aux",
                                                     name=f"den_{qb}")
                    nc.tensor.matmul(dens[qb], on8, pts[(qb, j)],
                                     start=(j == 0), stop=(j == NPAIR - 1),
                                     perf_mode=DR)

                def emit_recip(qb):
                    # ~18-bit approx: den is a well-conditioned positive sum,
                    # and a 4e-6 relative error on the softmax denominator is
                    # invisible next to the fp8 quantization noise.
                    rdb = rdp.tile([P, QB], F32, tag="rdb", name=f"rdb_{qb}")
                    nc.vector.reciprocal_approx_fast(rdb, dens[qb])
                    rdbs[qb] = rdb

                def emit_norm(qb):
                    at = atp.tile([P, 2, QB], FP8, tag="attn", name=f"at_{qb}")
                    pv0, pv1 = pvs[qb]
                    nc.vector.tensor_mul(at[:, 0, :], pv0, rdbs[qb])
                    nc.vector.tensor_mul(at[:, 1, :], pv1, rdbs[qb])
                    attns[qb] = at

                def emit_proj(qb, oc):
                    # output DMAs stay off the act-engine queue: a waiting
                    # descriptor competes with the exp stream's sequencer.
                    qs = slice(QB * qb, QB * (qb + 1))
                    po = pools["aux"].tile([P, QB], F32, tag="aux",
                                           name=f"po{qb}_{oc}")
                    nc.tensor.matmul(po, wpT8[:, :, oc * P:(oc + 1) * P],
                                     attns[qb], start=True, stop=True,
                                     perf_mode=DR)
                    outsb = otp.tile([P, QB], F32, tag="outsb")
                    nc.vector.scalar_tensor_tensor(
                        out=outsb, in0=po, scalar=1.0 / 32.0,
                        in1=Xq32r[:, oc, qs], op0=OP.mult, op1=OP.add)
                    (nc.sync if (2 * qb + oc) % 2 == 0 else
                     nc.gpsimd).dma_start(out=y_t[:, oc, qs], in_=outsb)

                # KS production (fp8 DoubleRow from X8) interleaved with
                # block 0's first pairs; mb0's casts ride the act engine so
                # the exp stream starts immediately behind them.
                with tc.tile_pool(name="ps_ks", bufs=2, space="PSUM") as psk:
                    def emit_ks(mb):
                        for co in range(2):
                            ks_ps = psk.tile([P, QB], F32, tag="ks")
                            nc.tensor.matmul(
                                ks_ps, M2Tf8[:, :, co * P:(co + 1) * P],
                                X8[:, :, QB * mb:QB * (mb + 1)],
                                start=True, stop=True, perf_mode=DR)
                            ksl = KS8[:, co, QB * mb:QB * (mb + 1)]
                            if mb < 2:
                                nc.scalar.activation(out=ksl, in_=ks_ps,
                                                     func=AF.Identity, bias=0.0)
                            else:
                                nc.vector.tensor_copy(ksl, ks_ps)

                    emit_ks(0)
                    emit_ks(1)
                    for p in range(6):
                        emit_ks(p + 2)
                        emit_s_exp(0, p)
                        if p >= 2:
                            emit_pv(0, p - 2)

                # schedule: exp stream never waits; PE work (PV/den) and all
                # DVE/projection work for block qb-1 hide inside block qb.
                with tc.tile_pool(name="ps_aux", bufs=2, space="PSUM") as psx:
                    pools["aux"] = psx
                    for qb in range(NQB):
                        for j in range(NPAIR):
                            if qb == 0 and j < 6:
                                continue  # emitted inside the KS scope
                            emit_s_exp(qb, j)
                            prev = qb - 1
                            if prev >= 0:
                                if j == 0:
                                    emit_pv(prev, NPAIR - 2)
                                    emit_den(prev, NPAIR - 2)
                                elif j == 1:
                                    emit_pv(prev, NPAIR - 1)
                                    emit_den(prev, NPAIR - 1)
                                elif j == 2:
                                    emit_recip(prev)
                                elif j == 3:
                                    emit_norm(prev)
                                elif j == 4:
                                    emit_proj(prev, 0)
                                elif j == 5:
                                    emit_proj(prev, 1)
                            if j == 6:
                                for jj in (range(4, 5) if qb == 0 else
                                           range(5)):
                                    emit_pv(qb, jj)
                            elif j == 7:
                                emit_pv(qb, 5)
                                for jj in range(6):
                                    emit_den(qb, jj)
                            elif 8 <= j <= 15:
                                emit_pv(qb, j - 2)
                                emit_den(qb, j - 2)
                            if j == 10 and qb + 1 < NQB:
                                emit_xq8(qb + 1)
                            elif j == 11:
                                emit_xq32r(qb)
                            if j == 15:
                                if qb == NQB - 1:
                                    # shorten the drain: pair 14's PV/den
                                    # can ride right behind its exp
                                    emit_pv(qb, NPAIR - 2)
                                    emit_den(qb, NPAIR - 2)

                    # drain: last block's tail in 256-wide slices so the
                    # normalize/project/store pipeline overlaps itself
                    qb = NQB - 1
                    emit_pv(qb, NPAIR - 1)
                    emit_den(qb, NPAIR - 1)
                    emit_recip(qb)
                    at = atp.tile([P, 2, QB], FP8, tag="attn", name="at_drain")
                    pv0, pv1 = pvs[qb]
                    for hh in range(2):
                        h = slice(256 * hh, 256 * (hh + 1))
                        nc.vector.tensor_mul(at[:, 0, h], pv0[:, h],
                                             rdbs[qb][:, h])
                        nc.vector.tensor_mul(at[:, 1, h], pv1[:, h],
                                             rdbs[qb][:, h])
                        for oc in range(2):
                            po = psx.tile([P, QB], F32, tag="aux",
                                          name=f"poD{hh}_{oc}")
                            nc.tensor.matmul(po[:, 0:256],
                                             wpT8[:, :, oc * P:(oc + 1) * P],
                                             at[:, :, h], start=True,
                                             stop=True, perf_mode=DR)
                            outsb = otp.tile([P, QB], F32, tag="outsb")
                            hq = slice(QB * qb + 256 * hh,
                                       QB * qb + 256 * (hh + 1))
                            nc.vector.scalar_tensor_tensor(
                                out=outsb[:, 0:256], in0=po[:, 0:256],
                                scalar=1.0 / 32.0, in1=Xq32r[:, oc, hq],
                                op0=OP.mult, op1=OP.add)
                            # sync-queue only: the gpsimd queue's end-of-
                            # kernel drain charges ~2.5us for a DMA still in
                            # flight there
                            nc.sync.dma_start(out=y_t[:, oc, hq],
                                              in_=outsb[:, 0:256])

    nc.compile()
    return nc


def _get_nc():
    if "nc" not in _cache:
        _cache["nc"] = _build()
    return _cache["nc"]


def _in_maps(inputs):
    """Per-core input maps. Core p = 2*b + h gets batch b's image with the
    key columns rotated so its query half sits at columns [0, NQ) -- keys
    are an unordered set under attention, so the rotation is free and saves
    a separate 2MB query load."""
    import ml_dtypes
    x = np.ascontiguousarray(np.asarray(inputs["x"], dtype=np.float32)
                             ).reshape(4, C, HW)
    common = {
        "consts": _pack_consts(inputs["gn_gamma"], inputs["gn_beta"],
                               inputs["bv"], inputs["bp"]),
        "ones8": _ones8(),
        "wq": np.asarray(inputs["wq"], np.float32),
        "wk": np.asarray(inputs["wk"], np.float32),
        "wv": np.asarray(inputs["wv"], np.float32),
        "wp": np.asarray(inputs["wp"], np.float32),
    }
    in_maps = []
    xb16 = [x[b].astype(ml_dtypes.bfloat16) for b in range(4)]
    for p in range(NCORES):
        b, h = divmod(p, 2)
        m = dict(common)
        m["xb"] = (xb16[b] if h == 0 else
                   np.ascontiguousarray(np.roll(xb16[b], -NQ, axis=1)))
        in_maps.append(m)
    return in_maps


def kernel(**inputs):
    from concourse.bass_utils import run_bass_kernel_spmd

    nc = _get_nc()
    res = run_bass_kernel_spmd(nc, _in_maps(inputs), list(range(NCORES)))
    out = np.empty((4, C, HW), np.float32)
    for p in range(NCORES):
        b, h = divmod(p, 2)
        out[b, :, h * NQ:(h + 1) * NQ] = res.results[p]["y"]
    return out.reshape(4, C, 64, 64)
